# revision 34
# baseline (speedup 1.0000x reference)
"""MQA attention kernel (B=2, T=2048, C=2048, 16 query heads, D=128, RoPE,
causal) for 8 Trainium2 NeuronCores.

Sharding: core = (batch, head-group-of-4). Each core computes q projection for
its 4 heads, the full shared K/V projection for its batch (MQA), causal
attention, and a partial output projection; the host sums the 4 partials per
batch.

Device layout notes:
  - Host pre-transposes x to xT [C, T] so all contractions over C read
    contiguous DRAM.
  - RoPE's even/odd interleave is turned into a half-split layout by permuting
    Wq/Wk columns on the host (scores are invariant to a shared permutation of
    the head dim).  Wq is also pre-scaled by 1/sqrt(D).
  - Scores are computed transposed, S.T[j, i], so the p@V and output
    projections need no on-chip transposes; softmax denominators come from an
    all-ones [128,128] stationary matmul (sums replicated across partitions),
    inverted with a fast-approx reciprocal and fused into the PSUM evacuation.
  - Matmuls run as float32r by default (full PE rate for moving dim >= 256,
    ~4e-4 rel error); KDT=bf16 switches to bf16 (~13% faster, ~6e-3 error).
"""

import os
import sys

if "/opt/trn_rl_repo" not in sys.path:
    sys.path.insert(0, "/opt/trn_rl_repo")

import numpy as np

import concourse.bacc as bacc
import concourse.mybir as mybir
import concourse.tile as tile
from concourse.bass_utils import run_bass_kernel_spmd

T = 2048
C = 2048
D = 128
N_HEAD = 16
HPC = 4  # heads per core
N_CORES = 8
F32 = mybir.dt.float32
F32R = mybir.dt.float32r
BF16 = mybir.dt.bfloat16
EXP = mybir.ActivationFunctionType.Exp


KDT = os.environ.get("KDT", "f32r")


def build_program():
    MD = BF16 if KDT == "bf16" else F32R
    AVD = BF16 if KDT in ("bf16", "mix") else F32R
    nc = bacc.Bacc("TRN2", target_bir_lowering=False, debug=False)

    xt = nc.dram_tensor("xt", [C, T], MD, kind="ExternalInput")
    wq = nc.dram_tensor("wq", [C, HPC * D], MD, kind="ExternalInput")
    wk = nc.dram_tensor("wk", [C, D], MD, kind="ExternalInput")
    wv = nc.dram_tensor("wv", [C, D], MD, kind="ExternalInput")
    wo = nc.dram_tensor("wo", [HPC * D, C], MD, kind="ExternalInput")
    cc = nc.dram_tensor("cc", [D, T], MD, kind="ExternalInput")
    ss = nc.dram_tensor("ss", [D, T], MD, kind="ExternalInput")
    ones_d = nc.dram_tensor("ones_d", [128, 128], AVD, kind="ExternalInput")
    ident_d = nc.dram_tensor("ident_d", [128, 128], MD, kind="ExternalInput")
    out = nc.dram_tensor("out", [T, C], F32, kind="ExternalOutput")

    xt_r = xt.rearrange("(ko p) t -> p ko t", p=128)
    wq_r = wq.rearrange("(ko p) m -> p ko m", p=128)
    wk_r = wk.rearrange("(ko p) m -> p ko m", p=128)
    wv_r = wv.rearrange("(ko p) m -> p ko m", p=128)
    wo_r = wo.rearrange("(ho p) c -> p ho c", p=128)
    out_r = out.rearrange("(mo p) c -> p mo c", p=128)

    with (
        tile.TileContext(nc) as tc,
        tc.tile_pool(name="consts", bufs=1) as consts,
        tc.tile_pool(name="qkpool", bufs=20) as qkpool,
        tc.tile_pool(name="ytpool", bufs=16) as ytpool,
        tc.tile_pool(name="wpool", bufs=1) as wpool,
    ):
        wqs = wpool.tile([128, 16, 512], MD, tag="w")
        for k in range(16):
            eng = [nc.scalar, nc.sync][k % 2]
            eng.dma_start(out=wqs[:, k, :], in_=wq_r[:, k, :])
        ident = consts.tile([128, 128], MD, tag="ident")
        nc.scalar.dma_start(out=ident, in_=ident_d[:, :])
        ones = consts.tile([128, 128], AVD, tag="ones")
        nc.scalar.dma_start(out=ones, in_=ones_d[:, :])
        # tri[j, i] = 1 if i >= j else 0 (keep causal-valid entries)
        tri = consts.tile([128, 128], BF16 if KDT in ("bf16", "mix") else F32, tag="tri")
        nc.gpsimd.memset(tri, 1.0)
        nc.gpsimd.affine_select(
            out=tri,
            in_=tri,
            compare_op=mybir.AluOpType.is_ge,
            fill=0.0,
            base=0,
            pattern=[[1, 128]],
            channel_multiplier=-1,
        )
        ccs = consts.tile([128, T], MD, tag="cc")
        nc.scalar.dma_start(out=ccs, in_=cc[:, :])
        sss = consts.tile([128, T], MD, tag="ss")
        nc.scalar.dma_start(out=sss, in_=ss[:, :])
        wks = consts.tile([128, 16, 128], MD, tag="wk")
        nc.scalar.dma_start(out=wks, in_=wk_r)
        wvs = consts.tile([128, 16, 128], MD, tag="wv")
        nc.scalar.dma_start(out=wvs, in_=wv_r)
        vsb = [
            consts.tile([128, 128], AVD, tag=f"vsb{j}", name=f"vsb{j}")
            for j in range(16)
        ]  # v, natural [t, d] per j-tile

        # qk[idx][c] = 512-wide chunk c of q.T (idx<4) / k.T (idx=4), RoPE'd
        qk = [
            [qkpool.tile([128, 512], MD, tag="qk", name=f"qk{i}_{c}") for c in range(4)]
            for i in range(5)
        ]
        yt = [
            [ytpool.tile([128, 512], MD, tag="yt", name=f"yt{i}_{c}") for c in range(4)]
            for i in range(4)
        ]

        # ---- phase 1: q/k/v projections (contraction over C) ----
        with (
            tc.tile_pool(name="t512", bufs=6) as t512,
            tc.tile_pool(name="psA", bufs=4, space="PSUM") as psA,
            tc.tile_pool(name="psY", bufs=2, space="PSUM") as psY,
            tc.tile_pool(name="psS", bufs=2, space="PSUM") as psS,
        ):
            for tcn in range(4):
                tsl = slice(tcn * 512, (tcn + 1) * 512)
                pq = [psA.tile([128, 512], F32, tag="ps", name=f"pq{i}") for i in range(4)]
                pk = psY.tile([128, 512], F32, tag="py", name=f"pk{tcn}")
                pv = psS.tile([128, 512], F32, tag="pss", name=f"pv{tcn}")
                for k in range(16):
                    xtt = t512.tile([128, 512], MD, tag="xt", name=f"xt{tcn}_{k}")
                    nc.sync.dma_start(out=xtt, in_=xt_r[:, k, tsl])
                    st, sp = k == 0, k == 15
                    for h in range(4):
                        nc.tensor.matmul(
                            pq[h],
                            wqs[:, k, h * 128 : (h + 1) * 128],
                            xtt,
                            start=st,
                            stop=sp,
                        )
                    nc.tensor.matmul(pk, wks[:, k, :], xtt, start=st, stop=sp)
                    nc.tensor.matmul(pv, wvs[:, k, :], xtt, start=st, stop=sp)
                # v natural tiles for this chunk via PE transpose
                vtt = t512.tile([128, 512], MD, tag="misc", name=f"vtt{tcn}")
                nc.scalar.copy(out=vtt, in_=pv)
                for mm in range(4):
                    m = tcn * 4 + mm
                    ptp = psA.tile([128, 512], MD, tag="ps", name=f"ptp{m}")
                    nc.tensor.transpose(
                        ptp[:, :128], vtt[:, mm * 128 : (mm + 1) * 128], ident
                    )
                    nc.scalar.copy(out=vsb[m], in_=ptp[:, :128])
                for h in range(4):
                    nc.scalar.copy(out=qk[h][tcn], in_=pq[h])
                nc.scalar.copy(out=qk[4][tcn], in_=pk)

                # RoPE this chunk (k first so attention unblocks earliest)
                for idx in [4, 0, 1, 2, 3]:
                    qc = qk[idx][tcn]
                    sw = t512.tile([128, 512], MD, tag="sw", name=f"sw{tcn}_{idx}")
                    nc.gpsimd.dma_start(out=sw[0:64, :], in_=qc[64:128, :])
                    nc.gpsimd.dma_start(out=sw[64:128, :], in_=qc[0:64, :])
                    nc.vector.tensor_mul(out=qc[:], in0=qc[:], in1=ccs[:, tsl])
                    nc.vector.tensor_mul(out=sw[:], in0=sw[:], in1=sss[:, tsl])
                    nc.vector.tensor_add(out=qc[:], in0=qc[:], in1=sw[:])

            # load Wo (reuses wq's slot; sync queue is idle after xt)
            wos = wpool.tile([128, 4, T], MD, tag="w")
            nc.sync.dma_start(out=wos, in_=wo_r)

            # ---- phase 2: causal attention, scores transposed S.T[j, i] ----
            for c in range(4):
                for h in range(4):
                    i0 = c * 512
                    py = psY.tile([128, 512], F32, tag="py")
                    psm = psS.tile([128, 512], F32, tag="pss")
                    njj = 4 * c + 4
                    pending = None
                    sums_started = False
                    for jj in range(njj):
                        r = jj - 4 * c  # >= 0 only for diagonal-group tiles
                        off = 128 * r if r >= 0 else 0
                        pss = psA.tile([128, 512], F32, tag="ps")
                        nc.tensor.matmul(
                            pss[:, off:],
                            qk[4][jj // 4][:, (jj % 4) * 128 : (jj % 4 + 1) * 128],
                            qk[h][c][:, off:],
                            start=True,
                            stop=True,
                        )
                        pT = t512.tile([128, 512], AVD, tag="pt")
                        nc.scalar.activation(out=pT[:, off:], in_=pss[:, off:], func=EXP)
                        if r >= 0:
                            nc.vector.tensor_mul(
                                out=pT[:, off : off + 128],
                                in0=pT[:, off : off + 128],
                                in1=tri,
                            )
                        nc.tensor.matmul(
                            py[:, off:],
                            vsb[jj],
                            pT[:, off:],
                            start=jj == 0,
                            stop=jj == njj - 1,
                        )
                        # denominator: accumulate all full (non-diag) tiles on
                        # DVE into one tile -> a single ones-matmul; diagonal
                        # tiles stream individually
                        if r < 0:
                            if pending is None:
                                pending = pT
                            else:
                                nc.vector.tensor_add(out=pending, in0=pending, in1=pT)
                            if jj == 4 * c - 1:  # last full tile
                                nc.tensor.matmul(
                                    psm,
                                    (ones),
                                    pending,
                                    start=not sums_started,
                                    stop=False,
                                )
                                sums_started = True
                                pending = None
                        else:
                            nc.tensor.matmul(
                                psm[:, off:],
                                (ones),
                                (pT[:, off:]),
                                start=not sums_started,
                                stop=jj == njj - 1,
                            )
                            sums_started = True
                    bc = t512.tile([128, 512], F32, tag="misc", name=f"bc{c}_{h}")
                    nc.vector.reciprocal_approx_fast(out=bc, in_=psm)
                    nc.vector.tensor_mul(out=yt[h][c], in0=py, in1=bc)


            # ---- phase 3: partial output projection (contraction over d) ----
            for m in range(16):
                for cn in range(4):
                    po = psA.tile([128, 512], F32, tag="ps")
                    for h in range(4):
                        nc.tensor.matmul(
                            po,
                            yt[h][m // 4][:, (m % 4) * 128 : (m % 4 + 1) * 128],
                            wos[:, h, cn * 512 : (cn + 1) * 512],
                            start=h == 0,
                            stop=h == 3,
                        )
                    ot = t512.tile([128, 512], F32, tag="misc")
                    nc.vector.tensor_copy(out=ot, in_=po)
                    nc.sync.dma_start(out=out_r[:, m, cn * 512 : (cn + 1) * 512], in_=ot)

    nc.compile()
    return nc


_PERM = np.concatenate([np.arange(0, D, 2), np.arange(1, D, 2)])

import ml_dtypes

DT_NP = ml_dtypes.bfloat16 if KDT == "bf16" else np.float32
AV_NP = ml_dtypes.bfloat16 if KDT in ("bf16", "mix") else np.float32


def make_in_maps(x, freqs_cos, freqs_sin, Wq, Wk, Wv, Wo):
    x = np.asarray(x, dtype=np.float32)
    freqs_cos = np.asarray(freqs_cos, dtype=np.float32)
    freqs_sin = np.asarray(freqs_sin, dtype=np.float32)
    Wq = np.asarray(Wq, dtype=np.float32)
    Wk = np.asarray(Wk, dtype=np.float32)
    Wv = np.asarray(Wv, dtype=np.float32)
    Wo = np.asarray(Wo, dtype=np.float32)

    scale = 1.0 / np.sqrt(np.float32(D))
    cosT = np.ascontiguousarray(freqs_cos.T)  # [64, T]
    sinT = np.ascontiguousarray(freqs_sin.T)
    cc = np.ascontiguousarray(np.concatenate([cosT, cosT], axis=0))  # [128, T]
    ss = np.ascontiguousarray(np.concatenate([-sinT, sinT], axis=0))
    wk_p = np.ascontiguousarray(Wk[:, _PERM])
    wv_c = np.ascontiguousarray(Wv)

    xts = [np.ascontiguousarray(x[b].T) for b in range(2)]

    ones_a = np.ones((128, 128), dtype=AV_NP)
    ident_a = np.eye(128, dtype=DT_NP)
    in_maps = []
    for core in range(N_CORES):
        b = core // 4
        hg = core % 4
        heads = range(4 * hg, 4 * hg + 4)
        qcols = np.concatenate([h * D + _PERM for h in heads])
        wq_c = np.ascontiguousarray(Wq[:, qcols] * scale)
        orows = np.concatenate([np.arange(h * D, (h + 1) * D) for h in heads])
        wo_c = np.ascontiguousarray(Wo[orows, :])
        in_maps.append(
            {
                "xt": xts[b].astype(DT_NP),
                "wq": wq_c.astype(DT_NP),
                "wk": wk_p.astype(DT_NP),
                "wv": wv_c.astype(DT_NP),
                "wo": wo_c.astype(DT_NP),
                "cc": cc.astype(DT_NP),
                "ss": ss.astype(DT_NP),
                "ones_d": ones_a,
                "ident_d": ident_a,
            }
        )
    return in_maps


_PROGRAM = None


def get_program():
    global _PROGRAM
    if _PROGRAM is None:
        _PROGRAM = build_program()
    return _PROGRAM


def kernel(x, freqs_cos, freqs_sin, Wq, Wk, Wv, Wo, _collect=None):
    nc = get_program()
    in_maps = make_in_maps(x, freqs_cos, freqs_sin, Wq, Wk, Wv, Wo)
    res = run_bass_kernel_spmd(nc, in_maps, core_ids=list(range(N_CORES)))
    if _collect is not None:
        _collect.append(res)
    outs = [r["out"] for r in res.results]
    full = np.empty((2, T, C), dtype=np.float32)
    for b in range(2):
        full[b] = outs[4 * b] + outs[4 * b + 1] + outs[4 * b + 2] + outs[4 * b + 3]
    return full


# revision 35
# speedup vs baseline: 1.0673x; 1.0673x over previous
"""MQA attention kernel (B=2, T=2048, C=2048, 16 query heads, D=128, RoPE,
causal) for 8 Trainium2 NeuronCores.

Sharding: core = (batch, head-group-of-4). Each core computes q projection for
its 4 heads, the full shared K/V projection for its batch (MQA), causal
attention, and a partial output projection; the host sums the 4 partials per
batch.

Device layout notes:
  - Host pre-transposes x to xT [C, T] so all contractions over C read
    contiguous DRAM.
  - RoPE's even/odd interleave is turned into a half-split layout by permuting
    Wq/Wk columns on the host (scores are invariant to a shared permutation of
    the head dim).  Wq is also pre-scaled by 1/sqrt(D).
  - Scores are computed transposed, S.T[j, i], so the p@V and output
    projections need no on-chip transposes; softmax denominators come from an
    all-ones [128,128] stationary matmul (sums replicated across partitions),
    inverted with a fast-approx reciprocal and fused into the PSUM evacuation.
  - Matmuls run as float32r by default (full PE rate for moving dim >= 256,
    ~4e-4 rel error); KDT=bf16 switches to bf16 (~13% faster, ~6e-3 error).
"""

import os
import sys

if "/opt/trn_rl_repo" not in sys.path:
    sys.path.insert(0, "/opt/trn_rl_repo")

import numpy as np

import concourse.bacc as bacc
import concourse.mybir as mybir
import concourse.tile as tile
from concourse.bass_utils import run_bass_kernel_spmd

T = 2048
C = 2048
D = 128
N_HEAD = 16
HPC = 4  # heads per core
N_CORES = 8
F32 = mybir.dt.float32
F32R = mybir.dt.float32r
BF16 = mybir.dt.bfloat16
EXP = mybir.ActivationFunctionType.Exp


KDT = os.environ.get("KDT", "f32r")


def build_program():
    MD = BF16 if KDT == "bf16" else F32R
    AVD = BF16 if KDT in ("bf16", "mix") else F32R
    nc = bacc.Bacc("TRN2", target_bir_lowering=False, debug=False)

    xt = nc.dram_tensor("xt", [C, T], MD, kind="ExternalInput")
    wq = nc.dram_tensor("wq", [C, HPC * D], MD, kind="ExternalInput")
    wk = nc.dram_tensor("wk", [C, D], MD, kind="ExternalInput")
    wv = nc.dram_tensor("wv", [C, D], MD, kind="ExternalInput")
    wo = nc.dram_tensor("wo", [HPC * D, C], MD, kind="ExternalInput")
    cc = nc.dram_tensor("cc", [D, T], MD, kind="ExternalInput")
    ss = nc.dram_tensor("ss", [D, T], MD, kind="ExternalInput")
    ones_d = nc.dram_tensor("ones_d", [128, 128], AVD, kind="ExternalInput")
    ident_d = nc.dram_tensor("ident_d", [128, 128], MD, kind="ExternalInput")
    out = nc.dram_tensor("out", [T, C], F32, kind="ExternalOutput")

    xt_r = xt.rearrange("(ko p) t -> p ko t", p=128)
    wq_r = wq.rearrange("(ko p) m -> p ko m", p=128)
    wk_r = wk.rearrange("(ko p) m -> p ko m", p=128)
    wv_r = wv.rearrange("(ko p) m -> p ko m", p=128)
    wo_r = wo.rearrange("(ho p) c -> p ho c", p=128)
    out_r = out.rearrange("(mo p) c -> p mo c", p=128)

    with (
        tile.TileContext(nc) as tc,
        tc.tile_pool(name="consts", bufs=1) as consts,
        tc.tile_pool(name="qkpool", bufs=20) as qkpool,
        tc.tile_pool(name="ytpool", bufs=16) as ytpool,
        tc.tile_pool(name="wpool", bufs=1) as wpool,
    ):
        wqs = wpool.tile([128, 16, 512], MD, tag="w")
        for k in range(16):
            eng = [nc.scalar, nc.sync][k % 2]
            eng.dma_start(out=wqs[:, k, :], in_=wq_r[:, k, :])
        ident = consts.tile([128, 128], MD, tag="ident")
        nc.scalar.dma_start(out=ident, in_=ident_d[:, :])
        ones = consts.tile([128, 128], AVD, tag="ones")
        nc.scalar.dma_start(out=ones, in_=ones_d[:, :])
        # tri[j, i] = 1 if i >= j else 0 (keep causal-valid entries)
        tri = consts.tile([128, 128], BF16 if KDT in ("bf16", "mix") else F32, tag="tri")
        nc.gpsimd.memset(tri, 1.0)
        nc.gpsimd.affine_select(
            out=tri,
            in_=tri,
            compare_op=mybir.AluOpType.is_ge,
            fill=0.0,
            base=0,
            pattern=[[1, 128]],
            channel_multiplier=-1,
        )
        ccs = consts.tile([128, T], MD, tag="cc")
        nc.scalar.dma_start(out=ccs, in_=cc[:, :])
        sss = consts.tile([128, T], MD, tag="ss")
        nc.scalar.dma_start(out=sss, in_=ss[:, :])
        wks = consts.tile([128, 16, 128], MD, tag="wk")
        nc.scalar.dma_start(out=wks, in_=wk_r)
        wvs = consts.tile([128, 16, 128], MD, tag="wv")
        nc.scalar.dma_start(out=wvs, in_=wv_r)
        vsb = [
            consts.tile([128, 128], AVD, tag=f"vsb{j}", name=f"vsb{j}")
            for j in range(16)
        ]  # v, natural [t, d] per j-tile

        # qk[idx][c] = 512-wide chunk c of q.T (idx<4) / k.T (idx=4), RoPE'd
        qk = [
            [qkpool.tile([128, 512], MD, tag="qk", name=f"qk{i}_{c}") for c in range(4)]
            for i in range(5)
        ]
        yt = [
            [ytpool.tile([128, 512], MD, tag="yt", name=f"yt{i}_{c}") for c in range(4)]
            for i in range(4)
        ]

        # ---- phase 1: q/k/v projections (contraction over C) ----
        with (
            tc.tile_pool(name="t512", bufs=6) as t512,
            tc.tile_pool(name="psA", bufs=4, space="PSUM") as psA,
            tc.tile_pool(name="psY", bufs=2, space="PSUM") as psY,
            tc.tile_pool(name="psS", bufs=2, space="PSUM") as psS,
        ):
            for tcn in range(4):
                tsl = slice(tcn * 512, (tcn + 1) * 512)
                pq = [psA.tile([128, 512], F32, tag="ps", name=f"pq{i}") for i in range(4)]
                pk = psY.tile([128, 512], F32, tag="py", name=f"pk{tcn}")
                pv = psS.tile([128, 512], F32, tag="pss", name=f"pv{tcn}")
                for k in range(16):
                    xtt = t512.tile([128, 512], MD, tag="xt", name=f"xt{tcn}_{k}")
                    nc.sync.dma_start(out=xtt, in_=xt_r[:, k, tsl])
                    st, sp = k == 0, k == 15
                    for h in range(4):
                        nc.tensor.matmul(
                            pq[h],
                            wqs[:, k, h * 128 : (h + 1) * 128],
                            xtt,
                            start=st,
                            stop=sp,
                        )
                    nc.tensor.matmul(pk, wks[:, k, :], xtt, start=st, stop=sp)
                    nc.tensor.matmul(pv, wvs[:, k, :], xtt, start=st, stop=sp)
                # v natural tiles for this chunk via PE transpose
                vtt = t512.tile([128, 512], MD, tag="misc", name=f"vtt{tcn}")
                nc.scalar.copy(out=vtt, in_=pv)
                for mm in range(4):
                    m = tcn * 4 + mm
                    ptp = psA.tile([128, 512], MD, tag="ps", name=f"ptp{m}")
                    nc.tensor.transpose(
                        ptp[:, :128], vtt[:, mm * 128 : (mm + 1) * 128], ident
                    )
                    nc.scalar.copy(out=vsb[m], in_=ptp[:, :128])
                for h in range(4):
                    nc.scalar.copy(out=qk[h][tcn], in_=pq[h])
                nc.scalar.copy(out=qk[4][tcn], in_=pk)

                # RoPE this chunk (k first so attention unblocks earliest)
                for idx in [4, 0, 1, 2, 3]:
                    qc = qk[idx][tcn]
                    sw = t512.tile([128, 512], MD, tag="sw", name=f"sw{tcn}_{idx}")
                    nc.gpsimd.dma_start(out=sw[0:64, :], in_=qc[64:128, :])
                    nc.gpsimd.dma_start(out=sw[64:128, :], in_=qc[0:64, :])
                    nc.vector.tensor_mul(out=qc[:], in0=qc[:], in1=ccs[:, tsl])
                    nc.vector.tensor_mul(out=sw[:], in0=sw[:], in1=sss[:, tsl])
                    nc.vector.tensor_add(out=qc[:], in0=qc[:], in1=sw[:])

            # load Wo (reuses wq's slot; sync queue is idle after xt)
            wos = wpool.tile([128, 4, T], MD, tag="w")
            nc.sync.dma_start(out=wos, in_=wo_r)

            # ---- phase 2: causal attention, scores transposed S.T[j, i] ----
            for c in range(4):
                for h in range(4):
                    i0 = c * 512
                    py = psY.tile([128, 512], F32, tag="py")
                    psm = psS.tile([128, 512], F32, tag="pss")
                    njj = 4 * c + 4
                    pending = None
                    sums_started = False
                    for jj in range(njj):
                        r = jj - 4 * c  # >= 0 only for diagonal-group tiles
                        off = 128 * r if r >= 0 else 0
                        pss = psA.tile([128, 512], F32, tag="ps")
                        nc.tensor.matmul(
                            pss[:, off:],
                            qk[4][jj // 4][:, (jj % 4) * 128 : (jj % 4 + 1) * 128],
                            qk[h][c][:, off:],
                            start=True,
                            stop=True,
                        )
                        pT = t512.tile([128, 512], AVD, tag="pt")
                        nc.scalar.activation(out=pT[:, off:], in_=pss[:, off:], func=EXP)
                        if r >= 0:
                            nc.vector.tensor_mul(
                                out=pT[:, off : off + 128],
                                in0=pT[:, off : off + 128],
                                in1=tri,
                            )
                        nc.tensor.matmul(
                            py[:, off:],
                            vsb[jj],
                            pT[:, off:],
                            start=jj == 0,
                            stop=jj == njj - 1,
                        )
                        # denominator: pair-sum full tiles on DVE to halve the
                        # extra PE stream; diagonal tiles go individually
                        if r < 0:
                            if pending is None:
                                pending = pT
                            else:
                                pts = t512.tile(
                                    [128, 512], AVD, tag="pts", name=f"pts{c}_{h}_{jj}"
                                )
                                nc.vector.tensor_add(out=pts, in0=pending, in1=pT)
                                nc.tensor.matmul(
                                    psm,
                                    (ones),
                                    pts,
                                    start=not sums_started,
                                    stop=False,
                                )
                                sums_started = True
                                pending = None
                        else:
                            nc.tensor.matmul(
                                psm[:, off:],
                                (ones),
                                (pT[:, off:]),
                                start=not sums_started,
                                stop=jj == njj - 1,
                            )
                            sums_started = True
                    bc = t512.tile([128, 512], F32, tag="misc", name=f"bc{c}_{h}")
                    nc.vector.reciprocal_approx_fast(out=bc, in_=psm)
                    nc.vector.tensor_mul(out=yt[h][c], in0=py, in1=bc)


            # ---- phase 3: partial output projection (contraction over d) ----
            for m in range(16):
                for cn in range(4):
                    po = psA.tile([128, 512], F32, tag="ps")
                    for h in range(4):
                        nc.tensor.matmul(
                            po,
                            yt[h][m // 4][:, (m % 4) * 128 : (m % 4 + 1) * 128],
                            wos[:, h, cn * 512 : (cn + 1) * 512],
                            start=h == 0,
                            stop=h == 3,
                        )
                    ot = t512.tile([128, 512], F32, tag="misc")
                    nc.vector.tensor_copy(out=ot, in_=po)
                    nc.sync.dma_start(out=out_r[:, m, cn * 512 : (cn + 1) * 512], in_=ot)

    nc.compile()
    return nc


_PERM = np.concatenate([np.arange(0, D, 2), np.arange(1, D, 2)])

import ml_dtypes

DT_NP = ml_dtypes.bfloat16 if KDT == "bf16" else np.float32
AV_NP = ml_dtypes.bfloat16 if KDT in ("bf16", "mix") else np.float32


def make_in_maps(x, freqs_cos, freqs_sin, Wq, Wk, Wv, Wo):
    x = np.asarray(x, dtype=np.float32)
    freqs_cos = np.asarray(freqs_cos, dtype=np.float32)
    freqs_sin = np.asarray(freqs_sin, dtype=np.float32)
    Wq = np.asarray(Wq, dtype=np.float32)
    Wk = np.asarray(Wk, dtype=np.float32)
    Wv = np.asarray(Wv, dtype=np.float32)
    Wo = np.asarray(Wo, dtype=np.float32)

    scale = 1.0 / np.sqrt(np.float32(D))
    cosT = np.ascontiguousarray(freqs_cos.T)  # [64, T]
    sinT = np.ascontiguousarray(freqs_sin.T)
    cc = np.ascontiguousarray(np.concatenate([cosT, cosT], axis=0))  # [128, T]
    ss = np.ascontiguousarray(np.concatenate([-sinT, sinT], axis=0))
    wk_p = np.ascontiguousarray(Wk[:, _PERM])
    wv_c = np.ascontiguousarray(Wv)

    xts = [np.ascontiguousarray(x[b].T) for b in range(2)]

    ones_a = np.ones((128, 128), dtype=AV_NP)
    ident_a = np.eye(128, dtype=DT_NP)
    in_maps = []
    for core in range(N_CORES):
        b = core // 4
        hg = core % 4
        heads = range(4 * hg, 4 * hg + 4)
        qcols = np.concatenate([h * D + _PERM for h in heads])
        wq_c = np.ascontiguousarray(Wq[:, qcols] * scale)
        orows = np.concatenate([np.arange(h * D, (h + 1) * D) for h in heads])
        wo_c = np.ascontiguousarray(Wo[orows, :])
        in_maps.append(
            {
                "xt": xts[b].astype(DT_NP),
                "wq": wq_c.astype(DT_NP),
                "wk": wk_p.astype(DT_NP),
                "wv": wv_c.astype(DT_NP),
                "wo": wo_c.astype(DT_NP),
                "cc": cc.astype(DT_NP),
                "ss": ss.astype(DT_NP),
                "ones_d": ones_a,
                "ident_d": ident_a,
            }
        )
    return in_maps


_PROGRAM = None


def get_program():
    global _PROGRAM
    if _PROGRAM is None:
        _PROGRAM = build_program()
    return _PROGRAM


def kernel(x, freqs_cos, freqs_sin, Wq, Wk, Wv, Wo, _collect=None):
    nc = get_program()
    in_maps = make_in_maps(x, freqs_cos, freqs_sin, Wq, Wk, Wv, Wo)
    res = run_bass_kernel_spmd(nc, in_maps, core_ids=list(range(N_CORES)))
    if _collect is not None:
        _collect.append(res)
    outs = [r["out"] for r in res.results]
    full = np.empty((2, T, C), dtype=np.float32)
    for b in range(2):
        full[b] = outs[4 * b] + outs[4 * b + 1] + outs[4 * b + 2] + outs[4 * b + 3]
    return full


# revision 36
# speedup vs baseline: 1.0810x; 1.0128x over previous
"""MQA attention kernel (B=2, T=2048, C=2048, 16 query heads, D=128, RoPE,
causal) for 8 Trainium2 NeuronCores.

Sharding: core = (batch, head-group-of-4). Each core computes q projection for
its 4 heads, the full shared K/V projection for its batch (MQA), causal
attention, and a partial output projection; the host sums the 4 partials per
batch.

Device layout notes:
  - Host pre-transposes x to xT [C, T] so all contractions over C read
    contiguous DRAM.
  - RoPE's even/odd interleave is turned into a half-split layout by permuting
    Wq/Wk columns on the host (scores are invariant to a shared permutation of
    the head dim).  Wq is also pre-scaled by 1/sqrt(D).
  - Scores are computed transposed, S.T[j, i], so the p@V and output
    projections need no on-chip transposes; softmax denominators come from an
    all-ones [128,128] stationary matmul (sums replicated across partitions),
    inverted with a fast-approx reciprocal and fused into the PSUM evacuation.
  - Matmuls run as float32r by default (full PE rate for moving dim >= 256,
    ~4e-4 rel error); KDT=bf16 switches to bf16 (~13% faster, ~6e-3 error).
"""

import os
import sys

if "/opt/trn_rl_repo" not in sys.path:
    sys.path.insert(0, "/opt/trn_rl_repo")

import numpy as np

import concourse.bacc as bacc
import concourse.mybir as mybir
import concourse.tile as tile
from concourse.bass_utils import run_bass_kernel_spmd

T = 2048
C = 2048
D = 128
N_HEAD = 16
HPC = 4  # heads per core
N_CORES = 8
F32 = mybir.dt.float32
F32R = mybir.dt.float32r
BF16 = mybir.dt.bfloat16
EXP = mybir.ActivationFunctionType.Exp


KDT = os.environ.get("KDT", "f32r")


def build_program():
    MD = BF16 if KDT == "bf16" else F32R
    AVD = BF16 if KDT in ("bf16", "mix") else F32R
    nc = bacc.Bacc("TRN2", target_bir_lowering=False, debug=False)

    xt = nc.dram_tensor("xt", [C, T], MD, kind="ExternalInput")
    wq = nc.dram_tensor("wq", [C, HPC * D], MD, kind="ExternalInput")
    wk = nc.dram_tensor("wk", [C, D], MD, kind="ExternalInput")
    wv = nc.dram_tensor("wv", [C, D], MD, kind="ExternalInput")
    wo = nc.dram_tensor("wo", [HPC * D, C], MD, kind="ExternalInput")
    cc = nc.dram_tensor("cc", [D, T], MD, kind="ExternalInput")
    ss = nc.dram_tensor("ss", [D, T], MD, kind="ExternalInput")
    ones_d = nc.dram_tensor("ones_d", [128, 128], AVD, kind="ExternalInput")
    ident_d = nc.dram_tensor("ident_d", [128, 128], MD, kind="ExternalInput")
    out = nc.dram_tensor("out", [T, C], F32, kind="ExternalOutput")

    xt_r = xt.rearrange("(ko p) t -> p ko t", p=128)
    wq_r = wq.rearrange("(ko p) m -> p ko m", p=128)
    wk_r = wk.rearrange("(ko p) m -> p ko m", p=128)
    wv_r = wv.rearrange("(ko p) m -> p ko m", p=128)
    wo_r = wo.rearrange("(ho p) c -> p ho c", p=128)
    out_r = out.rearrange("(mo p) c -> p mo c", p=128)

    with (
        tile.TileContext(nc) as tc,
        tc.tile_pool(name="consts", bufs=1) as consts,
        tc.tile_pool(name="qkpool", bufs=20) as qkpool,
        tc.tile_pool(name="ytpool", bufs=16) as ytpool,
        tc.tile_pool(name="wpool", bufs=1) as wpool,
    ):
        wqs = wpool.tile([128, 16, 512], MD, tag="w")
        for k in range(16):
            eng = [nc.scalar, nc.sync][k % 2]
            eng.dma_start(out=wqs[:, k, :], in_=wq_r[:, k, :])
        ident = consts.tile([128, 128], MD, tag="ident")
        nc.scalar.dma_start(out=ident, in_=ident_d[:, :])
        ones = consts.tile([128, 128], AVD, tag="ones")
        nc.scalar.dma_start(out=ones, in_=ones_d[:, :])
        # tri[j, i] = 1 if i >= j else 0 (keep causal-valid entries)
        tri = consts.tile([128, 128], BF16 if KDT in ("bf16", "mix") else F32, tag="tri")
        nc.gpsimd.memset(tri, 1.0)
        nc.gpsimd.affine_select(
            out=tri,
            in_=tri,
            compare_op=mybir.AluOpType.is_ge,
            fill=0.0,
            base=0,
            pattern=[[1, 128]],
            channel_multiplier=-1,
        )
        ccs = consts.tile([128, T], MD, tag="cc")
        nc.scalar.dma_start(out=ccs, in_=cc[:, :])
        sss = consts.tile([128, T], MD, tag="ss")
        nc.scalar.dma_start(out=sss, in_=ss[:, :])
        wks = consts.tile([128, 16, 128], MD, tag="wk")
        nc.scalar.dma_start(out=wks, in_=wk_r)
        wvs = consts.tile([128, 16, 128], MD, tag="wv")
        nc.scalar.dma_start(out=wvs, in_=wv_r)
        vsb = [
            consts.tile([128, 128], AVD, tag=f"vsb{j}", name=f"vsb{j}")
            for j in range(16)
        ]  # v, natural [t, d] per j-tile

        # qk[idx][c] = 512-wide chunk c of q.T (idx<4) / k.T (idx=4), RoPE'd
        qk = [
            [qkpool.tile([128, 512], MD, tag="qk", name=f"qk{i}_{c}") for c in range(4)]
            for i in range(5)
        ]
        yt = [
            [ytpool.tile([128, 512], MD, tag="yt", name=f"yt{i}_{c}") for c in range(4)]
            for i in range(4)
        ]

        # ---- phase 1: q/k/v projections (contraction over C) ----
        with (
            tc.tile_pool(name="t512", bufs=6) as t512,
            tc.tile_pool(name="psA", bufs=4, space="PSUM") as psA,
            tc.tile_pool(name="psY", bufs=2, space="PSUM") as psY,
            tc.tile_pool(name="psS", bufs=2, space="PSUM") as psS,
        ):
            for tcn in range(4):
                tsl = slice(tcn * 512, (tcn + 1) * 512)
                pq = [psA.tile([128, 512], F32, tag="ps", name=f"pq{i}") for i in range(4)]
                pk = psY.tile([128, 512], F32, tag="py", name=f"pk{tcn}")
                pv = psS.tile([128, 512], F32, tag="pss", name=f"pv{tcn}")
                for k in range(16):
                    xtt = t512.tile([128, 512], MD, tag="xt", name=f"xt{tcn}_{k}")
                    nc.sync.dma_start(out=xtt, in_=xt_r[:, k, tsl])
                    st, sp = k == 0, k == 15
                    for h in range(4):
                        nc.tensor.matmul(
                            pq[h],
                            wqs[:, k, h * 128 : (h + 1) * 128],
                            xtt,
                            start=st,
                            stop=sp,
                        )
                    nc.tensor.matmul(pk, wks[:, k, :], xtt, start=st, stop=sp)
                    nc.tensor.matmul(pv, wvs[:, k, :], xtt, start=st, stop=sp)
                # v natural tiles for this chunk via PE transpose
                vtt = t512.tile([128, 512], MD, tag="misc", name=f"vtt{tcn}")
                nc.scalar.copy(out=vtt, in_=pv)
                for mm in range(4):
                    m = tcn * 4 + mm
                    ptp = psA.tile([128, 512], MD, tag="ps", name=f"ptp{m}")
                    nc.tensor.transpose(
                        ptp[:, :128], vtt[:, mm * 128 : (mm + 1) * 128], ident
                    )
                    nc.scalar.copy(out=vsb[m], in_=ptp[:, :128])
                for h in range(4):
                    nc.scalar.copy(out=qk[h][tcn], in_=pq[h])
                nc.scalar.copy(out=qk[4][tcn], in_=pk)

                # RoPE this chunk (k first so attention unblocks earliest)
                for idx in [4, 0, 1, 2, 3]:
                    qc = qk[idx][tcn]
                    sw = t512.tile([128, 512], MD, tag="sw", name=f"sw{tcn}_{idx}")
                    nc.gpsimd.dma_start(out=sw[0:64, :], in_=qc[64:128, :])
                    nc.gpsimd.dma_start(out=sw[64:128, :], in_=qc[0:64, :])
                    nc.vector.tensor_mul(out=qc[:], in0=qc[:], in1=ccs[:, tsl])
                    nc.gpsimd.tensor_mul(out=sw[:], in0=sw[:], in1=sss[:, tsl])
                    nc.vector.tensor_add(out=qc[:], in0=qc[:], in1=sw[:])

            # load Wo (reuses wq's slot; sync queue is idle after xt)
            wos = wpool.tile([128, 4, T], MD, tag="w")
            nc.sync.dma_start(out=wos, in_=wo_r)

            # ---- phase 2: causal attention, scores transposed S.T[j, i] ----
            for c in range(4):
                for h in range(4):
                    i0 = c * 512
                    py = psY.tile([128, 512], F32, tag="py")
                    psm = psS.tile([128, 512], F32, tag="pss")
                    njj = 4 * c + 4
                    pending = None
                    sums_started = False
                    for jj in range(njj):
                        r = jj - 4 * c  # >= 0 only for diagonal-group tiles
                        off = 128 * r if r >= 0 else 0
                        pss = psA.tile([128, 512], F32, tag="ps")
                        nc.tensor.matmul(
                            pss[:, off:],
                            qk[4][jj // 4][:, (jj % 4) * 128 : (jj % 4 + 1) * 128],
                            qk[h][c][:, off:],
                            start=True,
                            stop=True,
                        )
                        pT = t512.tile([128, 512], AVD, tag="pt")
                        nc.scalar.activation(out=pT[:, off:], in_=pss[:, off:], func=EXP)
                        if r >= 0:
                            nc.vector.tensor_mul(
                                out=pT[:, off : off + 128],
                                in0=pT[:, off : off + 128],
                                in1=tri,
                            )
                        nc.tensor.matmul(
                            py[:, off:],
                            vsb[jj],
                            pT[:, off:],
                            start=jj == 0,
                            stop=jj == njj - 1,
                        )
                        # denominator: pair-sum full tiles on DVE to halve the
                        # extra PE stream; diagonal tiles go individually
                        if r < 0:
                            if pending is None:
                                pending = pT
                            else:
                                pts = t512.tile(
                                    [128, 512], AVD, tag="pts", name=f"pts{c}_{h}_{jj}"
                                )
                                nc.vector.tensor_add(out=pts, in0=pending, in1=pT)
                                nc.tensor.matmul(
                                    psm,
                                    (ones),
                                    pts,
                                    start=not sums_started,
                                    stop=False,
                                )
                                sums_started = True
                                pending = None
                        else:
                            nc.tensor.matmul(
                                psm[:, off:],
                                (ones),
                                (pT[:, off:]),
                                start=not sums_started,
                                stop=jj == njj - 1,
                            )
                            sums_started = True
                    bc = t512.tile([128, 512], F32, tag="misc", name=f"bc{c}_{h}")
                    nc.vector.reciprocal_approx_fast(out=bc, in_=psm)
                    nc.vector.tensor_mul(out=yt[h][c], in0=py, in1=bc)


            # ---- phase 3: partial output projection (contraction over d) ----
            for m in range(16):
                for cn in range(4):
                    po = psA.tile([128, 512], F32, tag="ps")
                    for h in range(4):
                        nc.tensor.matmul(
                            po,
                            yt[h][m // 4][:, (m % 4) * 128 : (m % 4 + 1) * 128],
                            wos[:, h, cn * 512 : (cn + 1) * 512],
                            start=h == 0,
                            stop=h == 3,
                        )
                    ot = t512.tile([128, 512], F32, tag="misc")
                    nc.vector.tensor_copy(out=ot, in_=po)
                    nc.sync.dma_start(out=out_r[:, m, cn * 512 : (cn + 1) * 512], in_=ot)

    nc.compile()
    return nc


_PERM = np.concatenate([np.arange(0, D, 2), np.arange(1, D, 2)])

import ml_dtypes

DT_NP = ml_dtypes.bfloat16 if KDT == "bf16" else np.float32
AV_NP = ml_dtypes.bfloat16 if KDT in ("bf16", "mix") else np.float32


def make_in_maps(x, freqs_cos, freqs_sin, Wq, Wk, Wv, Wo):
    x = np.asarray(x, dtype=np.float32)
    freqs_cos = np.asarray(freqs_cos, dtype=np.float32)
    freqs_sin = np.asarray(freqs_sin, dtype=np.float32)
    Wq = np.asarray(Wq, dtype=np.float32)
    Wk = np.asarray(Wk, dtype=np.float32)
    Wv = np.asarray(Wv, dtype=np.float32)
    Wo = np.asarray(Wo, dtype=np.float32)

    scale = 1.0 / np.sqrt(np.float32(D))
    cosT = np.ascontiguousarray(freqs_cos.T)  # [64, T]
    sinT = np.ascontiguousarray(freqs_sin.T)
    cc = np.ascontiguousarray(np.concatenate([cosT, cosT], axis=0))  # [128, T]
    ss = np.ascontiguousarray(np.concatenate([-sinT, sinT], axis=0))
    wk_p = np.ascontiguousarray(Wk[:, _PERM])
    wv_c = np.ascontiguousarray(Wv)

    xts = [np.ascontiguousarray(x[b].T) for b in range(2)]

    ones_a = np.ones((128, 128), dtype=AV_NP)
    ident_a = np.eye(128, dtype=DT_NP)
    in_maps = []
    for core in range(N_CORES):
        b = core // 4
        hg = core % 4
        heads = range(4 * hg, 4 * hg + 4)
        qcols = np.concatenate([h * D + _PERM for h in heads])
        wq_c = np.ascontiguousarray(Wq[:, qcols] * scale)
        orows = np.concatenate([np.arange(h * D, (h + 1) * D) for h in heads])
        wo_c = np.ascontiguousarray(Wo[orows, :])
        in_maps.append(
            {
                "xt": xts[b].astype(DT_NP),
                "wq": wq_c.astype(DT_NP),
                "wk": wk_p.astype(DT_NP),
                "wv": wv_c.astype(DT_NP),
                "wo": wo_c.astype(DT_NP),
                "cc": cc.astype(DT_NP),
                "ss": ss.astype(DT_NP),
                "ones_d": ones_a,
                "ident_d": ident_a,
            }
        )
    return in_maps


_PROGRAM = None


def get_program():
    global _PROGRAM
    if _PROGRAM is None:
        _PROGRAM = build_program()
    return _PROGRAM


def kernel(x, freqs_cos, freqs_sin, Wq, Wk, Wv, Wo, _collect=None):
    nc = get_program()
    in_maps = make_in_maps(x, freqs_cos, freqs_sin, Wq, Wk, Wv, Wo)
    res = run_bass_kernel_spmd(nc, in_maps, core_ids=list(range(N_CORES)))
    if _collect is not None:
        _collect.append(res)
    outs = [r["out"] for r in res.results]
    full = np.empty((2, T, C), dtype=np.float32)
    for b in range(2):
        full[b] = outs[4 * b] + outs[4 * b + 1] + outs[4 * b + 2] + outs[4 * b + 3]
    return full
